# revision 16
# baseline (speedup 1.0000x reference)
"""Trainium2 Bass kernel for causal MultiHeadAttention.

Problem: B=4, S=2048, H=16, D=64, DM=1024, fp32 I/O.
  qkv = x @ w_qkv ; causal softmax attention per head ; out = attn @ w_out

Sharding (8 cores): 4-way batch x 2-way heads. Core c handles batch c//2 and
heads (c%2)*8 .. +8. Each core computes a partial out-projection (its 512
attention channels x full w_out row-slice); the host sums the two head-half
partials per batch while unsharding.

Per-core dataflow (all bf16 matmul inputs, fp32 PSUM):
  xt = x[b].T (host)                                  [1024, 2048]
  qT,kT = (w_qk_slice).T-major proj:  lhsT=w tiles, rhs=xt   -> [512ch, 2048]
  v    = row-major proj:             lhsT=xt tiles, rhs=w_v  -> [2048, 512ch]
  scoresT[ki,qi] = kT.T @ qT   (K=64, two heads row-packed in PE array)
  probsT = exp(scoresT + causal_addmask)   (no max pass: |scores| <= ~6)
  outT[d,qi]  = v.T-contract: lhsT=[v[ki,d]|ones] (M=65; row 64 = softmax denom)
  out_headsT  = outT * (1/denom)  (partition_broadcast of recip rows)
  partial_out = out_headsT.T @ w_out_slice  (row-major psum -> sbuf -> HBM)
"""

import numpy as np

B, S, H, D = 4, 2048, 16, 64
DM = H * D          # 1024
NCORES = 8
HPC = H // 2        # 8 heads per core
CQ = HPC * D        # 512 channels per core
NEG = -1.0e9

_PROG_CACHE = {}


def build_program():
    import concourse.mybir as mybir
    import concourse.tile as tile
    from concourse import bacc

    dt = mybir.dt
    f32 = dt.float32
    bf16 = dt.bfloat16
    AF = mybir.ActivationFunctionType

    nc = bacc.Bacc(None)
    xt = nc.declare_dram_parameter("xt", [DM, S], bf16, isOutput=False)
    wqk = nc.declare_dram_parameter("wqk", [DM, 2 * CQ], bf16, isOutput=False)
    wv = nc.declare_dram_parameter("wv", [DM, CQ], bf16, isOutput=False)
    wo = nc.declare_dram_parameter("wo", [CQ, DM], bf16, isOutput=False)
    mask = nc.declare_dram_parameter("mask", [128, 4 * 512], f32, isOutput=False)
    out = nc.declare_dram_parameter("out", [S, DM], f32, isOutput=True)

    KT = DM // 128      # 8 contraction tiles over model dim
    NRT = S // 128      # 16 row tiles over sequence
    NRC = S // 512      # 4 row chunks over sequence
    NP = HPC // 2       # 4 head pairs per core
    NST = S // 512      # 4 query supertiles

    with tile.TileContext(nc) as tc:
        with (
            tc.tile_pool(name="persist", bufs=1) as pp,
            tc.tile_pool(name="probs", bufs=4) as probsp,
            tc.tile_pool(name="recip", bufs=2) as recipp,
            tc.tile_pool(name="ostage", bufs=3) as ostagep,
            tc.tile_pool(name="psmm", bufs=2, space="PSUM") as psmm,
            tc.tile_pool(name="pssc", bufs=2, space="PSUM") as pssc,
            tc.tile_pool(name="psout", bufs=2, space="PSUM") as psout,
        ):
            # ---- load inputs to SBUF ----
            xt_sb = []
            wqk_sb = []
            wv_sb = []
            for i in range(KT):
                t = pp.tile([128, S], bf16, tag=f"xt{i}", name=f"xt{i}")
                nc.sync.dma_start(out=t[:], in_=xt[128 * i:128 * (i + 1), :])
                xt_sb.append(t)
                t = pp.tile([128, 2 * CQ], bf16, tag=f"wqk{i}", name=f"wqk{i}")
                nc.sync.dma_start(out=t[:], in_=wqk[128 * i:128 * (i + 1), :])
                wqk_sb.append(t)
                t = pp.tile([128, CQ], bf16, tag=f"wv{i}", name=f"wv{i}")
                nc.sync.dma_start(out=t[:], in_=wv[128 * i:128 * (i + 1), :])
                wv_sb.append(t)
            wo_sb = []
            for c in range(CQ // 128):
                t = pp.tile([128, DM], bf16, tag=f"wo{c}", name=f"wo{c}")
                nc.sync.dma_start(out=t[:], in_=wo[128 * c:128 * (c + 1), :])
                wo_sb.append(t)
            mask_sb = pp.tile([128, 4 * 512], f32, tag="mask", name="mask")
            nc.sync.dma_start(out=mask_sb[:], in_=mask[:, :])

            # persistent activation tensors; v tiles hold 65 cols per head
            # (64 v-channels + a ones column so PV also accumulates the
            # softmax denominator into output row 64)
            qT = [pp.tile([128, S], bf16, tag=f"qT{p}", name=f"qT{p}") for p in range(NP)]
            kT = [pp.tile([128, S], bf16, tag=f"kT{p}", name=f"kT{p}") for p in range(NP)]
            v_rm = [pp.tile([128, HPC * 65], bf16, tag=f"v{rt}", name=f"v{rt}") for rt in range(NRT)]
            oT = [pp.tile([128, S], bf16, tag=f"oT{p}", name=f"oT{p}") for p in range(NP)]

            # ---- V projection (row-major): psum[rows, vcols] ----
            for rt in range(NRT):
                v_view = v_rm[rt].rearrange("p (h c) -> p h c", c=65)
                nc.vector.memset(v_view[:, :, 64:65], 1.0)
                ps = psmm.tile([128, 512], f32, tag="mm", name="mm")
                for kt in range(KT):
                    nc.tensor.matmul(
                        ps[:],
                        lhsT=xt_sb[kt][:, 128 * rt:128 * (rt + 1)],
                        rhs=wv_sb[kt][:],
                        start=(kt == 0),
                        stop=(kt == KT - 1),
                    )
                nc.vector.tensor_copy(
                    v_view[:, :, 0:64], ps.rearrange("p (h c) -> p h c", c=64)
                )

            for p in range(NP):
                # ---- Q,K projection for this pair (channel-major) ----
                for ct in (p, NP + p):  # q col tile then k col tile
                    dst = qT[p] if ct < NP else kT[p]
                    for rc in range(NRC):
                        ps = psmm.tile([128, 512], f32, tag="mm", name="mm")
                        for kt in range(KT):
                            nc.tensor.matmul(
                                ps[:],
                                lhsT=wqk_sb[kt][:, 128 * ct:128 * (ct + 1)],
                                rhs=xt_sb[kt][:, 512 * rc:512 * (rc + 1)],
                                start=(kt == 0),
                                stop=(kt == KT - 1),
                            )
                        nc.vector.tensor_copy(dst[:, 512 * rc:512 * (rc + 1)], ps[:])

                # ---- attention for this pair, query supertiles of 512 ----
                for st in range(NST):
                    out_ps = [
                        psout.tile([65, 512], f32, tag="o", name="o")
                        for _ in range(2)
                    ]
                    nkb = 4 * st + 4
                    for kb in range(nkb):
                        r = kb - 4 * st  # >=0: diagonal block, mask variant r
                        first, last = (kb == 0), (kb == nkb - 1)
                        sc = pssc.tile([128, 1024], f32, tag="sc", name="sc")
                        for hh in range(2):
                            base, lo = 512 * hh, 64 * hh
                            nc.tensor.matmul(
                                sc[:, base:base + 512],
                                lhsT=kT[p][lo:lo + 64, 128 * kb:128 * (kb + 1)],
                                rhs=qT[p][lo:lo + 64, 512 * st:512 * (st + 1)],
                                start=True,
                                stop=True,
                                tile_position=(lo, 0),
                            )
                        if r >= 0:
                            for hh in range(2):
                                base = 512 * hh
                                nc.vector.tensor_add(
                                    sc[:, base:base + 512],
                                    sc[:, base:base + 512],
                                    mask_sb[:, 512 * r:512 * (r + 1)],
                                )
                        pr = probsp.tile([128, 1024], bf16, tag="pr", name="pr")
                        nc.scalar.activation(pr[:], sc[:], AF.Exp)
                        for hh in range(2):
                            base = 512 * hh
                            h = 2 * p + hh
                            nc.tensor.matmul(
                                out_ps[hh][:, :],
                                lhsT=v_rm[kb][:, 65 * h:65 * h + 65],
                                rhs=pr[:, base:base + 512],
                                start=first,
                                stop=last,
                            )
                    # normalize: oT[p][:, st] = out * (1/denom-row) broadcast
                    for hh in range(2):
                        rc_sb = recipp.tile([65, 512], f32, tag="recip", name="recip")
                        rc0_sb = recipp.tile([1, 512], f32, tag="recip0", name="recip0")
                        bc_sb = recipp.tile([64, 512], f32, tag="bc", name="bc")
                        nc.vector.reciprocal(rc_sb[64:65, :], out_ps[hh][64:65, :])
                        # partition_broadcast ucode reads via GPSIMD cpu0's
                        # partition window: source must sit at partition 0
                        nc.sync.dma_start(out=rc0_sb[0:1, :], in_=rc_sb[64:65, :])
                        nc.gpsimd.partition_broadcast(
                            bc_sb[0:64, :], rc0_sb[0:1, :]
                        )
                        if hh == 0:
                            nc.vector.tensor_mul(
                                oT[p][0:64, 512 * st:512 * (st + 1)],
                                out_ps[hh][0:64, :],
                                bc_sb[:, :],
                            )
                        else:
                            hi_sb = recipp.tile([64, 512], bf16, tag="hi", name="hi")
                            nc.vector.tensor_mul(
                                hi_sb[:, :], out_ps[hh][0:64, :], bc_sb[:, :]
                            )
                            nc.sync.dma_start(
                                out=oT[p][64:128, 512 * st:512 * (st + 1)],
                                in_=hi_sb[:, :],
                            )

            # ---- out projection: partial[rows, oc] ----
            for rt in range(NRT):
                for o2 in range(2):
                    ps = psmm.tile([128, 512], f32, tag="mm", name="mm")
                    for c in range(4):
                        nc.tensor.matmul(
                            ps[:],
                            lhsT=oT[c][:, 128 * rt:128 * (rt + 1)],
                            rhs=wo_sb[c][:, 512 * o2:512 * (o2 + 1)],
                            start=(c == 0),
                            stop=(c == 3),
                        )
                    st_sb = ostagep.tile([128, 512], f32, tag="ostage", name="ostage")
                    nc.vector.tensor_copy(st_sb[:], ps[:])
                    nc.sync.dma_start(
                        out=out[128 * rt:128 * (rt + 1), 512 * o2:512 * (o2 + 1)],
                        in_=st_sb[:],
                    )
    nc.finalize()
    return nc


def get_program():
    if "nc" not in _PROG_CACHE:
        _PROG_CACHE["nc"] = build_program()
    return _PROG_CACHE["nc"]


def make_in_maps(x, w_qkv, w_out):
    import ml_dtypes

    bf = ml_dtypes.bfloat16
    x = np.asarray(x, dtype=np.float32)
    w_qkv = np.asarray(w_qkv, dtype=np.float32)
    w_out = np.asarray(w_out, dtype=np.float32)
    scale = float(D) ** -0.5
    # diagonal-block mask variants: variant r masks (128*r + p > j) for
    # ki = 128*(4*st+r)+p vs qi = 512*st+j
    p_idx = np.arange(128)[:, None]
    j_idx = np.arange(512)[None, :]
    mask = np.concatenate(
        [np.where(128 * r + p_idx > j_idx, NEG, 0.0) for r in range(4)], axis=1
    ).astype(np.float32)
    in_maps = []
    for c in range(NCORES):
        b, hh = c // 2, c % 2
        q0 = CQ * hh
        wq = (w_qkv[:, q0:q0 + CQ] * scale).astype(bf)
        wk = w_qkv[:, DM + q0:DM + q0 + CQ].astype(bf)
        in_maps.append(
            {
                "xt": np.ascontiguousarray(x[b].T).astype(bf),
                "wqk": np.concatenate([wq, wk], axis=1),
                "wv": w_qkv[:, 2 * DM + q0:2 * DM + q0 + CQ].astype(bf),
                "wo": w_out[q0:q0 + CQ, :].astype(bf),
                "mask": mask,
            }
        )
    return in_maps


def gather(results):
    outs = [np.asarray(results[c]["out"], dtype=np.float32) for c in range(NCORES)]
    return np.stack([outs[2 * b] + outs[2 * b + 1] for b in range(B)], axis=0)


def kernel(x, w_qkv, w_out):
    from concourse.bass_utils import run_bass_kernel_spmd

    nc = get_program()
    in_maps = make_in_maps(x, w_qkv, w_out)
    res = run_bass_kernel_spmd(nc, in_maps, list(range(NCORES)))
    return gather(res.results)
